# revision 58
# baseline (speedup 1.0000x reference)
import os
import numpy as np
import ml_dtypes

B, H, N, D = 4, 12, 8192, 64
M = 128
NCORES = 8
PAIRS = (B * H) // NCORES  # 6
NH = N // 2                # 4096 columns per stacked half
NT = N // 128              # 64 chunks of 128 rows
NGRP = 3                   # NS groups of 2 pairs
GP = 2

LAST_RESULTS = None
_cache = {}


def _build():
    if "nc" in _cache:
        return _cache["nc"]
    import concourse.bacc as bacc
    import concourse.mybir as mybir
    import concourse.tile as tile

    f32, f16, bf16 = mybir.dt.float32, mybir.dt.float16, mybir.dt.bfloat16
    AF = mybir.ActivationFunctionType
    MUL, ADD = mybir.AluOpType.mult, mybir.AluOpType.add

    nc = bacc.Bacc("TRN2", target_bir_lowering=False, debug=False)
    QTT = nc.declare_dram_parameter("QTT", [PAIRS, 128, NH], f16, isOutput=False)
    KTT = nc.declare_dram_parameter("KTT", [PAIRS, 128, NH], f16, isOutput=False)
    VA = nc.declare_dram_parameter("VA", [PAIRS, 128, NT * 65], bf16, isOutput=False)
    NR2 = nc.declare_dram_parameter("NR2", [PAIRS, 128, M], f16, isOutput=False)
    NC2 = nc.declare_dram_parameter("NC2", [PAIRS, 128, M], f16, isOutput=False)
    K2 = nc.declare_dram_parameter("K2", [PAIRS, 128, M], f16, isOutput=False)
    GS = nc.declare_dram_parameter("GS", [1, 1], f32, isOutput=False)
    XOT = nc.declare_dram_parameter("XOT", [PAIRS, 65, 2, NH], bf16, isOutput=True)

    with tile.TileContext(nc) as tc:
        with (tc.tile_pool(name="cst", bufs=1) as cst,
              tc.tile_pool(name="pq", bufs=1) as pq,
              tc.tile_pool(name="pk", bufs=3) as pk,
              tc.tile_pool(name="pv", bufs=3) as pv,
              tc.tile_pool(name="sm2", bufs=3) as sm2,
              tc.tile_pool(name="per", bufs=1) as per,
              tc.tile_pool(name="erb", bufs=6) as erb,
              tc.tile_pool(name="nsb", bufs=2) as nsb,
              tc.tile_pool(name="ecb", bufs=16) as ecb,
              tc.tile_pool(name="xob", bufs=2) as xob):
            pse = tc.alloc_tile_pool(name="psAe", bufs=2, space="PSUM")
            pss = tc.alloc_tile_pool(name="psAs", bufs=1, space="PSUM")
            psn = tc.alloc_tile_pool(name="psAn", bufs=2, space="PSUM")

            # ---- constants ----
            i325 = cst.tile([128, GP, 128], f32, tag="i325")
            nc.gpsimd.memset(i325[:], 0.0)
            for j in range(GP):
                nc.gpsimd.affine_select(out=i325[:, j, :], in_=i325[:, j, :],
                    compare_op=mybir.AluOpType.not_equal, fill=3.25, base=0,
                    pattern=[[-1, 128]], channel_multiplier=1)
            dgt = cst.tile([128, 128], f16, tag="dgt")
            nc.gpsimd.memset(dgt[:], 0.0)
            nc.gpsimd.affine_select(out=dgt[:], in_=dgt[:],
                compare_op=mybir.AluOpType.not_equal, fill=-15.0 / 7.0, base=0,
                pattern=[[-1, 128]], channel_multiplier=1)
            ones_row = cst.tile([1, 128], f32, tag="ones_row")
            nc.vector.memset(ones_row[:], 1.0)
            gs_sb = cst.tile([1, 1], f32, tag="gs_sb")
            nc.sync.dma_start(gs_sb[:], GS[:])
            ps_bc = psn.tile([128, 1], f32, tag="ns")
            nc.tensor.matmul(ps_bc[:], ones_row[:], gs_sb[:], start=True, stop=True)
            gsb = cst.tile([128, 1], f32, tag="gsb")
            nc.vector.tensor_copy(gsb[:], ps_bc[:])

            Ypk = [cst.tile([128, GP, 128], f16, tag=f"y0g{g}", name=f"y0g{g}")
                   for g in range(NGRP)]
            Ppk = [cst.tile([128, GP, 64], f16, tag=f"p0g{g}", name=f"p0g{g}")
                   for g in range(NGRP)]

            qtt = [None] * PAIRS
            nc2t = [None] * PAIRS
            ns_state = {"Y": Ypk, "P": list(Ppk)}
            bsb = [None] * PAIRS

            def stage_a_all():
                # one continuous er-mm/exp stream across ALL pairs; S-matmuls
                # lag 3 blocks globally; next pair's head DMAs issue mid-pair
                st = {}

                def head(p):
                    ktt = pk.tile([128, NH], f16, tag="ktt", name="ktt")
                    if p == 0:
                        nc.sync.dma_start(ktt[0:64, :], KTT[p, 0:64, :])
                        nc.sync.dma_start(ktt[64:128, :], KTT[p, 64:128, :])
                    else:
                        nc.sync.dma_start(ktt[:], KTT[p])
                    nr2 = pk.tile([128, M], f16, tag="nr2", name="nr2")
                    nc.sync.dma_start(nr2[:], NR2[p])
                    k2sb = per.tile([128, M], f16, tag=f"k2{p}", name=f"k2{p}")
                    nc.sync.dma_start(k2sb[:], K2[p])
                    va = pv.tile([128, NT, 65], bf16, tag="va", name="va")
                    nc.sync.dma_start(va[:],
                                      VA[p].rearrange("pp (t d) -> pp t d", d=65))
                    ps_saA = pss.tile([128, 65], f32, tag="saccA", name="psA")
                    ps_saB = pss.tile([128, 65], f32, tag="saccB", name="psB")
                    st[p] = dict(ktt=ktt, va=va, nr2=nr2, k2sb=k2sb,
                                 ps_saA=ps_saA, ps_saB=ps_saB,
                                 ers=[None] * 8)

                def defer_prefix(p):
                    g, jg = p // GP, p % GP
                    k2sb = st[p]["k2sb"]
                    ps_y0 = psn.tile([128, M], f32, tag="ns", name="ps_y0")
                    nc.tensor.matmul(ps_y0[:], k2sb[:], k2sb[:],
                                     start=True, stop=True)
                    nc.vector.tensor_scalar_mul(Ypk[g][:, jg, :], ps_y0[:],
                                                gsb[:])

                def er_block(p, blk):
                    ktt, nr2 = st[p]["ktt"], st[p]["nr2"]
                    ps_e = pse.tile([128, 8, 128], f32, tag="er", name="ps_e")
                    for c8 in range(8):
                        c = blk * 8 + c8
                        half = (c & 1) * 64
                        col = (c >> 1) * 128
                        slot = (c8 & 1) * 4 + (c8 >> 1)
                        nc.tensor.matmul(ps_e[:, slot, :],
                            ktt[half:half + 64, col:col + 128],
                            nr2[half:half + 64, :], start=True, stop=True)
                    ert = erb.tile([128, 8, 128], bf16, tag="erb", name="ert")
                    nc.scalar.activation(ert[:], ps_e[:], AF.Exp)
                    st[p]["ers"][blk] = ert

                def s_block(p, blk):
                    # each chunk split into two K=64 row-group halves into
                    # separate PSUM banks: concurrent MMs + hideable LDWs
                    for c8 in range(8):
                        c = blk * 8 + c8
                        t_idx = (c & 1) * 32 + (c >> 1)
                        slot = (c8 & 1) * 4 + (c8 >> 1)
                        ert = st[p]["ers"][blk]
                        va = st[p]["va"]
                        nc.tensor.matmul(st[p]["ps_saA"][:],
                            ert[0:64, slot, :], va[0:64, t_idx, :],
                            start=(c == 0), stop=(c == 63),
                            skip_group_check=True)
                        nc.tensor.matmul(st[p]["ps_saB"][:],
                            ert[64:128, slot, :], va[64:128, t_idx, :],
                            start=(c == 0), stop=(c == 63),
                            skip_group_check=True)
                    st[p]["ers"][blk] = None

                def suffix(p):
                    g, jg = p // GP, p % GP
                    ns_state[f"adone{p}"] = True
                    k2sb = st[p]["k2sb"]
                    ssum = sm2.tile([128, 65], f32, tag="ssum", name="ssum")
                    nc.vector.tensor_copy(ssum[:], st[p]["ps_saA"][:])
                    nc.vector.tensor_tensor(out=ssum[:], in0=ssum[:],
                                            in1=st[p]["ps_saB"][:], op=ADD)
                    rr = sm2.tile([128, 1], f32, tag="rr", name="rr")
                    nc.vector.reciprocal(rr[:], ssum[:, 64:65])
                    rr2 = sm2.tile([128, 1], f32, tag="rr2", name="rr2")
                    nc.vector.tensor_tensor(out=rr2[:], in0=rr[:], in1=gsb[:],
                                            op=MUL)
                    s_bf = sm2.tile([128, 64], f16, tag="sbf", name="sbf")
                    nc.vector.tensor_scalar_mul(s_bf[:], ssum[:, 0:64], rr2[:])
                    ps_p0 = psn.tile([128, 64], f32, tag="ns", name="ps_p0")
                    nc.tensor.matmul(ps_p0[:], k2sb[:], s_bf[:],
                                     start=True, stop=True)
                    nc.vector.tensor_copy(Ppk[g][:, jg, :], ps_p0[:])

                LAG = 3
                head(0)
                blocks = [(p, b) for p in range(PAIRS) for b in range(8)]
                done_s = 0
                for idx, (p, b) in enumerate(blocks):
                    er_block(p, b)
                    if b == 1:
                        defer_prefix(p)
                    if b == 4 and p + 1 < PAIRS:
                        head(p + 1)
                    if p == 5 and b == 2:
                        c_prefetch(0)
                    if idx >= LAG:
                        sp, sb = blocks[done_s]
                        s_block(sp, sb)
                        done_s += 1
                        if sb == 7:
                            suffix(sp)
                    yield
                while done_s < len(blocks):
                    sp, sb = blocks[done_s]
                    s_block(sp, sb)
                    done_s += 1
                    if sb == 7:
                        suffix(sp)
                    yield

            def stage_b(g, psB):
                Ycur = ns_state["Y"][g]
                Pcur = ns_state["P"][g]
                for it in range(6):
                    ps_a = psB.tile([128, GP, 128], f32, tag="ns", name=f"psa{g}")
                    for j in range(GP):
                        nc.tensor.matmul(ps_a[:, j, :], Ycur[:, j, :],
                                         Ycur[:, j, :], start=True, stop=True)
                    yield
                    Rt = nsb.tile([128, GP, 128], f16, tag=f"R{g}", name=f"R{g}")
                    nc.vector.scalar_tensor_tensor(out=Rt[:], in0=ps_a[:],
                        scalar=-1.0 / 7.0, in1=Ycur[:], op0=MUL, op1=ADD)
                    yield
                    ps_q = psB.tile([128, GP, 128], f32, tag="ns", name=f"psq{g}")
                    for j in range(GP):
                        nc.tensor.matmul(ps_q[:, j, :], Ycur[:, j, :],
                                         Rt[:, j, :], start=True, stop=False)
                        nc.tensor.matmul(ps_q[:, j, :], dgt[:],
                                         Ycur[:, j, :], start=False, stop=True)
                    yield
                    qt_ = nsb.tile([128, GP, 128], f16, tag=f"q{g}", name=f"qm{g}")
                    nc.vector.scalar_tensor_tensor(out=qt_[:], in0=ps_q[:],
                        scalar=1.75, in1=i325[:], op0=MUL, op1=ADD)
                    yield
                    if it < 5:
                        ps_y = psB.tile([128, GP, 128], f32, tag="ns")
                        for j in range(GP):
                            nc.tensor.matmul(ps_y[:, j, :], qt_[:, j, :],
                                             Ycur[:, j, :], start=True, stop=True)
                        Ynew = nsb.tile([128, GP, 128], f16, tag=f"yn{g}",
                                        name=f"yn{g}")
                        nc.vector.tensor_copy(Ynew[:], ps_y[:])
                        Ycur = Ynew
                        yield
                    ps_p = psB.tile([128, GP, 64], f32, tag="ns")
                    for j in range(GP):
                        nc.tensor.matmul(ps_p[:, j, :], qt_[:, j, :],
                                         Pcur[:, j, :], start=True, stop=True)
                    Pnew = nsb.tile([128, GP, 64], f16, tag=f"pn{g}", name=f"pn{g}")
                    nc.vector.tensor_copy(Pnew[:], ps_p[:])
                    Pcur = Pnew
                    yield
                ns_state["P"][g] = Pcur
                for pp in range(GP):
                    p = g * GP + pp
                    bsb[p] = per.tile([128, 65], bf16, tag=f"b{p}", name=f"b{p}")
                    nc.vector.memset(bsb[p][:, 64:65], 1.0)
                    nc.vector.tensor_copy(bsb[p][:, 0:64], Pcur[:, pp, :])
                yield

            ecs_all = {}

            def c_prefetch(p):
                qtt[p] = pq.tile([128, NH], f16, tag=f"qt{p}", name=f"qt{p}")
                nc.sync.dma_start(qtt[p][:], QTT[p])
                nc2t[p] = pq.tile([128, M], f16, tag=f"nc2{p}", name=f"nc2{p}")
                nc.sync.dma_start(nc2t[p][:], NC2[p])

            def stage_c_pre(p, psE, delay=0):
                for _ in range(delay):
                    yield
                if p + 1 < PAIRS:
                    c_prefetch(p + 1)
                ecs = [None] * 8
                ecs_all[p] = ecs
                for sc in range(8):
                    ps_c = psE.tile([128, 2, 512], f32, tag="ec")
                    for t in range(2):
                        half = t * 64
                        nc.tensor.matmul(ps_c[:, t, :],
                            nc2t[p][half:half + 64, :],
                            qtt[p][half:half + 64, sc * 512:sc * 512 + 512],
                            start=True, stop=True)
                    ecs[sc] = ecb.tile([128, 2, 512], bf16, tag="ec", name="ect")
                    nc.scalar.activation(ecs[sc][:], ps_c[:], AF.Exp)
                    yield

            def stage_c_post(p, psx, act_share=0):
                ecs = ecs_all[p]
                xos = xob.tile([65, 2, NH], bf16, tag="xo")
                for sc in range(8):
                    ps_x = psx.tile([65, 2, 512], f32, tag="px", name="ps_x")
                    for t in range(2):
                        nc.tensor.matmul(ps_x[:, t, :], bsb[p][:],
                                         ecs[sc][:, t, :], start=True, stop=True)
                    dst = xos[:, :, sc * 512:sc * 512 + 512]
                    if sc >= 8 - act_share:
                        nc.scalar.copy(dst, ps_x[:])
                    else:
                        nc.vector.tensor_copy(dst, ps_x[:])
                    ecs[sc] = None
                    if sc == 3:
                        nc.sync.dma_start(XOT[p, :, :, 0:2048],
                                          xos[:, :, 0:2048])
                    yield
                nc.sync.dma_start(XOT[p, :, :, 2048:4096], xos[:, :, 2048:4096])
                yield

            def drain(*gens):
                live = list(gens)
                while live:
                    for gx in list(live):
                        try:
                            next(gx)
                        except StopIteration:
                            live.remove(gx)

            # schedule: A streams with B(0)/B(1) NS ladders zipped in as
            # their seed pairs complete; B(2) overlaps the first C-pre
            a = stage_a_all()
            live = [a]
            started = set()
            while live:
                for gx in list(live):
                    try:
                        next(gx)
                    except StopIteration:
                        live.remove(gx)
                for g in (0, 1):
                    if g not in started and ns_state.get(f"adone{g * GP + 1}"):
                        live.append(stage_b(g, psn))
                        started.add(g)
            assert started == {0, 1}
            psn.release()
            pss.release()
            pse.release()
            psE = tc.alloc_tile_pool(name="psE", bufs=2, space="PSUM")
            psB = tc.alloc_tile_pool(name="psB", bufs=2, space="PSUM")
            drain(stage_b(2, psB), stage_c_pre(0, psE, delay=4))
            psB.release()
            psx = tc.alloc_tile_pool(name="psx", bufs=2, space="PSUM")
            drain(stage_c_post(0, psx), stage_c_pre(1, psE))
            drain(stage_c_post(1, psx), stage_c_pre(2, psE))
            drain(stage_c_post(2, psx), stage_c_pre(3, psE))
            drain(stage_c_post(3, psx), stage_c_pre(4, psE))
            drain(stage_c_post(4, psx, act_share=2), stage_c_pre(5, psE))
            drain(stage_c_post(5, psx, act_share=6))
            psx.release()
            psE.release()
    nc.finalize()
    _cache["nc"] = nc
    return nc


def kernel(Q, K, V, mask):
    from concourse.bass_utils import run_bass_kernel_spmd

    Q = np.asarray(Q, dtype=np.float32)
    K = np.asarray(K, dtype=np.float32)
    V = np.asarray(V, dtype=np.float32)
    Qf = Q.reshape(B * H, N, D)
    Kf = K.reshape(B * H, N, D)
    Vf = V.reshape(B * H, N, D)

    nr = np.empty((B * H, M, D), np.float32)
    nc_ = np.empty((B * H, M, D), np.float32)
    K2h = np.empty((B * H, M, M), np.float16)
    gmax = 0.0
    for i in range(B * H):
        for (T, out) in ((Kf, nc_), (Qf, nr)):
            s = T[i, :, 0].copy()
            s[0] = np.inf
            idx = np.argpartition(-s, M)[:M]
            out[i] = T[i, np.sort(idx), :]
        m = nr[i].astype(np.float64) @ nc_[i].astype(np.float64).T
        e = np.exp(m - m.max(axis=1, keepdims=True))
        k2 = e / e.sum(axis=1, keepdims=True)
        K2h[i] = k2.astype(np.float16)
        gmax = max(gmax, float(k2.sum(axis=0).max()))

    def stack_halves(t16):  # [64, N] -> [128, N/2]
        return np.concatenate([t16[:, :NH], t16[:, NH:]], axis=0)

    QTTh = np.empty((B * H, 128, NH), np.float16)
    KTTh = np.empty((B * H, 128, NH), np.float16)
    for i in range(B * H):
        QTTh[i] = stack_halves(np.ascontiguousarray(Qf[i].T).astype(np.float16))
        KTTh[i] = stack_halves(np.ascontiguousarray(Kf[i].T).astype(np.float16))
    bf = ml_dtypes.bfloat16
    VAh = np.empty((B * H, 128, NT * 65), bf)
    ones = np.ones((N, 1), np.float32)
    for i in range(B * H):
        vaug = np.concatenate([Vf[i], ones], axis=1).astype(bf)  # [N, 65]
        VAh[i] = vaug.reshape(NT, 128, 65).transpose(1, 0, 2).reshape(128, NT * 65)
    nrt16 = np.ascontiguousarray(nr.transpose(0, 2, 1)).astype(np.float16)
    nct16 = np.ascontiguousarray(nc_.transpose(0, 2, 1)).astype(np.float16)
    NR2h = np.concatenate([nrt16, nrt16], axis=1)
    NC2h = np.concatenate([nct16, nct16], axis=1)
    gsv = np.array([[1.0 / gmax]], np.float32)

    ncb = _build()
    in_maps = []
    for c in range(NCORES):
        sl = slice(c * PAIRS, (c + 1) * PAIRS)
        in_maps.append({"QTT": QTTh[sl], "KTT": KTTh[sl], "VA": VAh[sl],
                        "NR2": NR2h[sl], "NC2": NC2h[sl],
                        "K2": K2h[sl], "GS": gsv})
    trace = os.environ.get("KERNEL_TRACE", "0") == "1"
    tmpdir = os.environ.get("KERNEL_TRACE_DIR") or None
    res = run_bass_kernel_spmd(ncb, in_maps, list(range(NCORES)),
                               trace=trace, tmpdir=tmpdir)
    global LAST_RESULTS
    LAST_RESULTS = res

    X = np.empty((B * H, N, D), np.float32)
    for c in range(NCORES):
        xot = np.asarray(res.results[c]["XOT"], dtype=np.float32)
        for pp in range(PAIRS):
            i = c * PAIRS + pp
            xf = xot[pp].reshape(65, N)
            X[i] = (xf[:64, :] / xf[64:65, :]).T
    return X.reshape(B, H, N, D)



# revision 60
# speedup vs baseline: 1.0638x; 1.0638x over previous
import os
import numpy as np
import ml_dtypes

B, H, N, D = 4, 12, 8192, 64
M = 128
NCORES = 8
PAIRS = (B * H) // NCORES  # 6
NH = N // 2                # 4096 columns per stacked half
NT = N // 128              # 64 chunks of 128 rows
NGRP = 3                   # NS groups of 2 pairs
GP = 2

LAST_RESULTS = None
_cache = {}


def _build():
    if "nc" in _cache:
        return _cache["nc"]
    import concourse.bacc as bacc
    import concourse.mybir as mybir
    import concourse.tile as tile

    f32, f16, bf16 = mybir.dt.float32, mybir.dt.float16, mybir.dt.bfloat16
    AF = mybir.ActivationFunctionType
    MUL, ADD = mybir.AluOpType.mult, mybir.AluOpType.add

    nc = bacc.Bacc("TRN2", target_bir_lowering=False, debug=False)
    QTT = nc.declare_dram_parameter("QTT", [PAIRS, 128, NH], f16, isOutput=False)
    KTT = nc.declare_dram_parameter("KTT", [PAIRS, 128, NH], f16, isOutput=False)
    VA = nc.declare_dram_parameter("VA", [PAIRS, 128, NT * 65], bf16, isOutput=False)
    NR2 = nc.declare_dram_parameter("NR2", [PAIRS, 128, M], f16, isOutput=False)
    NC2 = nc.declare_dram_parameter("NC2", [PAIRS, 128, M], f16, isOutput=False)
    K2 = nc.declare_dram_parameter("K2", [PAIRS, 128, M], f16, isOutput=False)
    GS = nc.declare_dram_parameter("GS", [1, 1], f32, isOutput=False)
    XOT = nc.declare_dram_parameter("XOT", [PAIRS, 65, 2, NH], bf16, isOutput=True)

    with tile.TileContext(nc) as tc:
        with (tc.tile_pool(name="cst", bufs=1) as cst,
              tc.tile_pool(name="pq", bufs=1) as pq,
              tc.tile_pool(name="pk", bufs=3) as pk,
              tc.tile_pool(name="pv", bufs=3) as pv,
              tc.tile_pool(name="sm2", bufs=3) as sm2,
              tc.tile_pool(name="per", bufs=1) as per,
              tc.tile_pool(name="erb", bufs=6) as erb,
              tc.tile_pool(name="nsb", bufs=2) as nsb,
              tc.tile_pool(name="ecb", bufs=16) as ecb,
              tc.tile_pool(name="xob", bufs=2) as xob):
            pse = tc.alloc_tile_pool(name="psAe", bufs=2, space="PSUM")
            pss = tc.alloc_tile_pool(name="psAs", bufs=2, space="PSUM")
            psn = tc.alloc_tile_pool(name="psAn", bufs=2, space="PSUM")

            # ---- constants ----
            i325 = cst.tile([128, GP, 128], f32, tag="i325")
            nc.gpsimd.memset(i325[:], 0.0)
            for j in range(GP):
                nc.gpsimd.affine_select(out=i325[:, j, :], in_=i325[:, j, :],
                    compare_op=mybir.AluOpType.not_equal, fill=3.25, base=0,
                    pattern=[[-1, 128]], channel_multiplier=1)
            dgt = cst.tile([128, 128], f16, tag="dgt")
            nc.gpsimd.memset(dgt[:], 0.0)
            nc.gpsimd.affine_select(out=dgt[:], in_=dgt[:],
                compare_op=mybir.AluOpType.not_equal, fill=-15.0 / 7.0, base=0,
                pattern=[[-1, 128]], channel_multiplier=1)
            ones_row = cst.tile([1, 128], f32, tag="ones_row")
            nc.vector.memset(ones_row[:], 1.0)
            gs_sb = cst.tile([1, 1], f32, tag="gs_sb")
            nc.sync.dma_start(gs_sb[:], GS[:])
            ps_bc = psn.tile([128, 1], f32, tag="ns")
            nc.tensor.matmul(ps_bc[:], ones_row[:], gs_sb[:], start=True, stop=True)
            gsb = cst.tile([128, 1], f32, tag="gsb")
            nc.vector.tensor_copy(gsb[:], ps_bc[:])

            Ypk = [cst.tile([128, GP, 128], f16, tag=f"y0g{g}", name=f"y0g{g}")
                   for g in range(NGRP)]
            Ppk = [cst.tile([128, GP, 64], f16, tag=f"p0g{g}", name=f"p0g{g}")
                   for g in range(NGRP)]

            qtt = [None] * PAIRS
            nc2t = [None] * PAIRS
            ns_state = {"Y": Ypk, "P": list(Ppk)}
            bsb = [None] * PAIRS

            def stage_a_all():
                # one continuous er-mm/exp stream across ALL pairs; S-matmuls
                # lag 3 blocks globally; next pair's head DMAs issue mid-pair
                st = {}

                def head(p):
                    ktt = pk.tile([128, NH], f16, tag="ktt", name="ktt")
                    if p == 0:
                        nc.sync.dma_start(ktt[0:64, :], KTT[p, 0:64, :])
                        nc.sync.dma_start(ktt[64:128, :], KTT[p, 64:128, :])
                    else:
                        nc.sync.dma_start(ktt[:], KTT[p])
                    nr2 = pk.tile([128, M], f16, tag="nr2", name="nr2")
                    nc.sync.dma_start(nr2[:], NR2[p])
                    k2sb = per.tile([128, M], f16, tag=f"k2{p}", name=f"k2{p}")
                    nc.sync.dma_start(k2sb[:], K2[p])
                    va = pv.tile([128, NT, 65], bf16, tag="va", name="va")
                    nc.sync.dma_start(va[:],
                                      VA[p].rearrange("pp (t d) -> pp t d", d=65))
                    ps_sa = pss.tile([128, 65], f32, tag="sacc", name="ps_sa")
                    st[p] = dict(ktt=ktt, va=va, nr2=nr2, k2sb=k2sb,
                                 ps_sa=ps_sa, ers=[None] * 8)

                def defer_prefix(p):
                    g, jg = p // GP, p % GP
                    k2sb = st[p]["k2sb"]
                    ps_y0 = psn.tile([128, M], f32, tag="ns", name="ps_y0")
                    nc.tensor.matmul(ps_y0[:], k2sb[:], k2sb[:],
                                     start=True, stop=True)
                    nc.vector.tensor_scalar_mul(Ypk[g][:, jg, :], ps_y0[:],
                                                gsb[:])

                def er_block(p, blk):
                    ktt, nr2 = st[p]["ktt"], st[p]["nr2"]
                    ps_e = pse.tile([128, 8, 128], f32, tag="er", name="ps_e")
                    for c8 in range(8):
                        c = blk * 8 + c8
                        half = (c & 1) * 64
                        col = (c >> 1) * 128
                        slot = (c8 & 1) * 4 + (c8 >> 1)
                        nc.tensor.matmul(ps_e[:, slot, :],
                            ktt[half:half + 64, col:col + 128],
                            nr2[half:half + 64, :], start=True, stop=True)
                    ert = erb.tile([128, 8, 128], bf16, tag="erb", name="ert")
                    nc.scalar.activation(ert[:], ps_e[:], AF.Exp)
                    st[p]["ers"][blk] = ert

                def s_block(p, blk):
                    for c8 in range(8):
                        c = blk * 8 + c8
                        t_idx = (c & 1) * 32 + (c >> 1)
                        slot = (c8 & 1) * 4 + (c8 >> 1)
                        nc.tensor.matmul(st[p]["ps_sa"][:],
                            st[p]["ers"][blk][:, slot, :], st[p]["va"][:, t_idx, :],
                            start=(c == 0), stop=(c == 63),
                            skip_group_check=True)
                    st[p]["ers"][blk] = None

                def suffix(p):
                    g, jg = p // GP, p % GP
                    ns_state[f"adone{p}"] = True
                    ps_sa, k2sb = st[p]["ps_sa"], st[p]["k2sb"]
                    rr = sm2.tile([128, 1], f32, tag="rr", name="rr")
                    nc.vector.reciprocal(rr[:], ps_sa[:, 64:65])
                    rr2 = sm2.tile([128, 1], f32, tag="rr2", name="rr2")
                    nc.vector.tensor_tensor(out=rr2[:], in0=rr[:], in1=gsb[:],
                                            op=MUL)
                    s_bf = sm2.tile([128, 64], f16, tag="sbf", name="sbf")
                    nc.vector.tensor_scalar_mul(s_bf[:], ps_sa[:, 0:64], rr2[:])
                    ps_p0 = psn.tile([128, 64], f32, tag="ns", name="ps_p0")
                    nc.tensor.matmul(ps_p0[:], k2sb[:], s_bf[:],
                                     start=True, stop=True)
                    nc.vector.tensor_copy(Ppk[g][:, jg, :], ps_p0[:])

                LAG = 3
                head(0)
                blocks = [(p, b) for p in range(PAIRS) for b in range(8)]
                done_s = 0
                for idx, (p, b) in enumerate(blocks):
                    er_block(p, b)
                    if b == 1:
                        defer_prefix(p)
                    if b == 4 and p + 1 < PAIRS:
                        head(p + 1)
                    if p == 5 and b == 2:
                        c_prefetch(0)
                    if idx >= LAG:
                        sp, sb = blocks[done_s]
                        s_block(sp, sb)
                        done_s += 1
                        if sb == 7:
                            suffix(sp)
                    yield
                while done_s < len(blocks):
                    sp, sb = blocks[done_s]
                    s_block(sp, sb)
                    done_s += 1
                    if sb == 7:
                        suffix(sp)
                    yield

            def stage_b(g, psB):
                Ycur = ns_state["Y"][g]
                Pcur = ns_state["P"][g]
                for it in range(6):
                    ps_a = psB.tile([128, GP, 128], f32, tag="ns", name=f"psa{g}")
                    for j in range(GP):
                        nc.tensor.matmul(ps_a[:, j, :], Ycur[:, j, :],
                                         Ycur[:, j, :], start=True, stop=True)
                    yield
                    Rt = nsb.tile([128, GP, 128], f16, tag=f"R{g}", name=f"R{g}")
                    nc.vector.scalar_tensor_tensor(out=Rt[:], in0=ps_a[:],
                        scalar=-1.0 / 7.0, in1=Ycur[:], op0=MUL, op1=ADD)
                    yield
                    ps_q = psB.tile([128, GP, 128], f32, tag="ns", name=f"psq{g}")
                    for j in range(GP):
                        nc.tensor.matmul(ps_q[:, j, :], Ycur[:, j, :],
                                         Rt[:, j, :], start=True, stop=False)
                        nc.tensor.matmul(ps_q[:, j, :], dgt[:],
                                         Ycur[:, j, :], start=False, stop=True)
                    yield
                    qt_ = nsb.tile([128, GP, 128], f16, tag=f"q{g}", name=f"qm{g}")
                    nc.vector.scalar_tensor_tensor(out=qt_[:], in0=ps_q[:],
                        scalar=1.75, in1=i325[:], op0=MUL, op1=ADD)
                    yield
                    if it < 5:
                        ps_y = psB.tile([128, GP, 128], f32, tag="ns")
                        for j in range(GP):
                            nc.tensor.matmul(ps_y[:, j, :], qt_[:, j, :],
                                             Ycur[:, j, :], start=True, stop=True)
                        Ynew = nsb.tile([128, GP, 128], f16, tag=f"yn{g}",
                                        name=f"yn{g}")
                        nc.vector.tensor_copy(Ynew[:], ps_y[:])
                        Ycur = Ynew
                        yield
                    ps_p = psB.tile([128, GP, 64], f32, tag="ns")
                    for j in range(GP):
                        nc.tensor.matmul(ps_p[:, j, :], qt_[:, j, :],
                                         Pcur[:, j, :], start=True, stop=True)
                    Pnew = nsb.tile([128, GP, 64], f16, tag=f"pn{g}", name=f"pn{g}")
                    nc.vector.tensor_copy(Pnew[:], ps_p[:])
                    Pcur = Pnew
                    yield
                ns_state["P"][g] = Pcur
                for pp in range(GP):
                    p = g * GP + pp
                    bsb[p] = per.tile([128, 65], bf16, tag=f"b{p}", name=f"b{p}")
                    nc.vector.memset(bsb[p][:, 64:65], 1.0)
                    nc.vector.tensor_copy(bsb[p][:, 0:64], Pcur[:, pp, :])
                yield

            ecs_all = {}

            def c_prefetch(p):
                qtt[p] = pq.tile([128, NH], f16, tag=f"qt{p}", name=f"qt{p}")
                nc.sync.dma_start(qtt[p][:], QTT[p])
                nc2t[p] = pq.tile([128, M], f16, tag=f"nc2{p}", name=f"nc2{p}")
                nc.sync.dma_start(nc2t[p][:], NC2[p])

            def stage_c_pre(p, psE, delay=0):
                for _ in range(delay):
                    yield
                if p + 1 < PAIRS:
                    c_prefetch(p + 1)
                ecs = [None] * 8
                ecs_all[p] = ecs
                for sc in range(8):
                    ps_c = psE.tile([128, 2, 512], f32, tag="ec")
                    for t in range(2):
                        half = t * 64
                        nc.tensor.matmul(ps_c[:, t, :],
                            nc2t[p][half:half + 64, :],
                            qtt[p][half:half + 64, sc * 512:sc * 512 + 512],
                            start=True, stop=True)
                    ecs[sc] = ecb.tile([128, 2, 512], bf16, tag="ec", name="ect")
                    nc.scalar.activation(ecs[sc][:], ps_c[:], AF.Exp)
                    yield

            def stage_c_post(p, psx, act_share=0):
                ecs = ecs_all[p]
                xos = xob.tile([65, 2, NH], bf16, tag="xo")
                for sc in range(8):
                    ps_x = psx.tile([65, 2, 512], f32, tag="px", name="ps_x")
                    for t in range(2):
                        nc.tensor.matmul(ps_x[:, t, :], bsb[p][:],
                                         ecs[sc][:, t, :], start=True, stop=True)
                    dst = xos[:, :, sc * 512:sc * 512 + 512]
                    if sc >= 8 - act_share:
                        nc.scalar.copy(dst, ps_x[:])
                    else:
                        nc.vector.tensor_copy(dst, ps_x[:])
                    ecs[sc] = None
                    if sc == 3:
                        nc.sync.dma_start(XOT[p, :, :, 0:2048],
                                          xos[:, :, 0:2048])
                    yield
                nc.sync.dma_start(XOT[p, :, :, 2048:4096], xos[:, :, 2048:4096])
                yield

            def drain(*gens):
                live = list(gens)
                while live:
                    for gx in list(live):
                        try:
                            next(gx)
                        except StopIteration:
                            live.remove(gx)

            # schedule: A streams with B(0)/B(1) NS ladders zipped in as
            # their seed pairs complete; B(2) overlaps the first C-pre
            a = stage_a_all()
            live = [a]
            started = set()
            while live:
                for gx in list(live):
                    try:
                        next(gx)
                    except StopIteration:
                        live.remove(gx)
                for g in (0, 1):
                    if g not in started and ns_state.get(f"adone{g * GP + 1}"):
                        live.append(stage_b(g, psn))
                        started.add(g)
            assert started == {0, 1}
            psn.release()
            pss.release()
            pse.release()
            psE = tc.alloc_tile_pool(name="psE", bufs=2, space="PSUM")
            psB = tc.alloc_tile_pool(name="psB", bufs=2, space="PSUM")
            drain(stage_b(2, psB), stage_c_pre(0, psE, delay=4))
            psB.release()
            psx = tc.alloc_tile_pool(name="psx", bufs=2, space="PSUM")
            drain(stage_c_post(0, psx), stage_c_pre(1, psE))
            drain(stage_c_post(1, psx), stage_c_pre(2, psE))
            drain(stage_c_post(2, psx), stage_c_pre(3, psE))
            drain(stage_c_post(3, psx), stage_c_pre(4, psE))
            drain(stage_c_post(4, psx, act_share=2), stage_c_pre(5, psE),
                  stage_c_post(5, psx, act_share=6))
            psx.release()
            psE.release()
    nc.finalize()
    _cache["nc"] = nc
    return nc


def kernel(Q, K, V, mask):
    from concourse.bass_utils import run_bass_kernel_spmd

    Q = np.asarray(Q, dtype=np.float32)
    K = np.asarray(K, dtype=np.float32)
    V = np.asarray(V, dtype=np.float32)
    Qf = Q.reshape(B * H, N, D)
    Kf = K.reshape(B * H, N, D)
    Vf = V.reshape(B * H, N, D)

    nr = np.empty((B * H, M, D), np.float32)
    nc_ = np.empty((B * H, M, D), np.float32)
    K2h = np.empty((B * H, M, M), np.float16)
    gmax = 0.0
    for i in range(B * H):
        for (T, out) in ((Kf, nc_), (Qf, nr)):
            s = T[i, :, 0].copy()
            s[0] = np.inf
            idx = np.argpartition(-s, M)[:M]
            out[i] = T[i, np.sort(idx), :]
        m = nr[i].astype(np.float64) @ nc_[i].astype(np.float64).T
        e = np.exp(m - m.max(axis=1, keepdims=True))
        k2 = e / e.sum(axis=1, keepdims=True)
        K2h[i] = k2.astype(np.float16)
        gmax = max(gmax, float(k2.sum(axis=0).max()))

    def stack_halves(t16):  # [64, N] -> [128, N/2]
        return np.concatenate([t16[:, :NH], t16[:, NH:]], axis=0)

    QTTh = np.empty((B * H, 128, NH), np.float16)
    KTTh = np.empty((B * H, 128, NH), np.float16)
    for i in range(B * H):
        QTTh[i] = stack_halves(np.ascontiguousarray(Qf[i].T).astype(np.float16))
        KTTh[i] = stack_halves(np.ascontiguousarray(Kf[i].T).astype(np.float16))
    bf = ml_dtypes.bfloat16
    VAh = np.empty((B * H, 128, NT * 65), bf)
    ones = np.ones((N, 1), np.float32)
    for i in range(B * H):
        vaug = np.concatenate([Vf[i], ones], axis=1).astype(bf)  # [N, 65]
        VAh[i] = vaug.reshape(NT, 128, 65).transpose(1, 0, 2).reshape(128, NT * 65)
    nrt16 = np.ascontiguousarray(nr.transpose(0, 2, 1)).astype(np.float16)
    nct16 = np.ascontiguousarray(nc_.transpose(0, 2, 1)).astype(np.float16)
    NR2h = np.concatenate([nrt16, nrt16], axis=1)
    NC2h = np.concatenate([nct16, nct16], axis=1)
    gsv = np.array([[1.0 / gmax]], np.float32)

    ncb = _build()
    in_maps = []
    for c in range(NCORES):
        sl = slice(c * PAIRS, (c + 1) * PAIRS)
        in_maps.append({"QTT": QTTh[sl], "KTT": KTTh[sl], "VA": VAh[sl],
                        "NR2": NR2h[sl], "NC2": NC2h[sl],
                        "K2": K2h[sl], "GS": gsv})
    trace = os.environ.get("KERNEL_TRACE", "0") == "1"
    tmpdir = os.environ.get("KERNEL_TRACE_DIR") or None
    res = run_bass_kernel_spmd(ncb, in_maps, list(range(NCORES)),
                               trace=trace, tmpdir=tmpdir)
    global LAST_RESULTS
    LAST_RESULTS = res

    X = np.empty((B * H, N, D), np.float32)
    for c in range(NCORES):
        xot = np.asarray(res.results[c]["XOT"], dtype=np.float32)
        for pp in range(PAIRS):
            i = c * PAIRS + pp
            xf = xot[pp].reshape(65, N)
            X[i] = (xf[:64, :] / xf[64:65, :]).T
    return X.reshape(B, H, N, D)



# revision 65
# speedup vs baseline: 1.0994x; 1.0334x over previous
import os
import numpy as np
import ml_dtypes

B, H, N, D = 4, 12, 8192, 64
M = 128
NCORES = 8
PAIRS = (B * H) // NCORES  # 6
NH = N // 2                # 4096 columns per stacked half
NT = N // 128              # 64 chunks of 128 rows
NGRP = 3                   # NS groups of 2 pairs
GP = 2

LAST_RESULTS = None
_cache = {}


def _build():
    if "nc" in _cache:
        return _cache["nc"]
    import concourse.bacc as bacc
    import concourse.mybir as mybir
    import concourse.tile as tile

    f32, f16, bf16 = mybir.dt.float32, mybir.dt.float16, mybir.dt.bfloat16
    AF = mybir.ActivationFunctionType
    MUL, ADD = mybir.AluOpType.mult, mybir.AluOpType.add

    nc = bacc.Bacc("TRN2", target_bir_lowering=False, debug=False)
    QTT = nc.declare_dram_parameter("QTT", [PAIRS, 128, NH], f16, isOutput=False)
    KTT = nc.declare_dram_parameter("KTT", [PAIRS, 128, NH], f16, isOutput=False)
    VA = nc.declare_dram_parameter("VA", [PAIRS, 128, NT * 65], bf16, isOutput=False)
    NR2 = nc.declare_dram_parameter("NR2", [PAIRS, 128, M], f16, isOutput=False)
    NC2 = nc.declare_dram_parameter("NC2", [PAIRS, 128, M], f16, isOutput=False)
    K2 = nc.declare_dram_parameter("K2", [PAIRS, 128, M], f16, isOutput=False)
    GS = nc.declare_dram_parameter("GS", [1, 1], f32, isOutput=False)
    XOT = nc.declare_dram_parameter("XOT", [PAIRS, 65, 2, NH], bf16, isOutput=True)

    with tile.TileContext(nc) as tc:
        with (tc.tile_pool(name="cst", bufs=1) as cst,
              tc.tile_pool(name="pq", bufs=1) as pq,
              tc.tile_pool(name="pk", bufs=3) as pk,
              tc.tile_pool(name="pv", bufs=3) as pv,
              tc.tile_pool(name="sm2", bufs=3) as sm2,
              tc.tile_pool(name="per", bufs=1) as per,
              tc.tile_pool(name="erb", bufs=6) as erb,
              tc.tile_pool(name="nsb", bufs=2) as nsb,
              tc.tile_pool(name="ecb", bufs=16) as ecb,
              tc.tile_pool(name="xob", bufs=2) as xob):
            pse = tc.alloc_tile_pool(name="psAe", bufs=2, space="PSUM")
            pss = tc.alloc_tile_pool(name="psAs", bufs=2, space="PSUM")
            psn = tc.alloc_tile_pool(name="psAn", bufs=2, space="PSUM")

            # ---- constants ----
            i325 = cst.tile([128, GP, 128], f32, tag="i325")
            nc.gpsimd.memset(i325[:], 0.0)
            for j in range(GP):
                nc.gpsimd.affine_select(out=i325[:, j, :], in_=i325[:, j, :],
                    compare_op=mybir.AluOpType.not_equal, fill=3.25, base=0,
                    pattern=[[-1, 128]], channel_multiplier=1)
            dgt = cst.tile([128, 128], f16, tag="dgt")
            nc.gpsimd.memset(dgt[:], 0.0)
            nc.gpsimd.affine_select(out=dgt[:], in_=dgt[:],
                compare_op=mybir.AluOpType.not_equal, fill=-15.0 / 7.0, base=0,
                pattern=[[-1, 128]], channel_multiplier=1)
            ones_row = cst.tile([1, 128], f32, tag="ones_row")
            nc.vector.memset(ones_row[:], 1.0)
            gs_sb = cst.tile([1, 1], f32, tag="gs_sb")
            nc.sync.dma_start(gs_sb[:], GS[:])
            ps_bc = psn.tile([128, 1], f32, tag="ns")
            nc.tensor.matmul(ps_bc[:], ones_row[:], gs_sb[:], start=True, stop=True)
            gsb = cst.tile([128, 1], f32, tag="gsb")
            nc.vector.tensor_copy(gsb[:], ps_bc[:])

            Ypk = [cst.tile([128, GP, 128], f16, tag=f"y0g{g}", name=f"y0g{g}")
                   for g in range(NGRP)]
            Ppk = [cst.tile([128, GP, 64], f16, tag=f"p0g{g}", name=f"p0g{g}")
                   for g in range(NGRP)]

            qtt = [None] * PAIRS
            nc2t = [None] * PAIRS
            ns_state = {"Y": Ypk, "P": list(Ppk)}
            bsb = [None] * PAIRS

            def stage_a_all():
                # one continuous er-mm/exp stream across ALL pairs; S-matmuls
                # lag 3 blocks globally; next pair's head DMAs issue mid-pair
                st = {}

                def head(p):
                    ktt = pk.tile([128, NH], f16, tag="ktt", name="ktt")
                    if p == 0:
                        nc.sync.dma_start(ktt[0:64, :], KTT[p, 0:64, :])
                        nc.sync.dma_start(ktt[64:128, :], KTT[p, 64:128, :])
                    else:
                        nc.sync.dma_start(ktt[:], KTT[p])
                    nr2 = pk.tile([128, M], f16, tag="nr2", name="nr2")
                    nc.sync.dma_start(nr2[:], NR2[p])
                    k2sb = per.tile([128, M], f16, tag=f"k2{p}", name=f"k2{p}")
                    nc.sync.dma_start(k2sb[:], K2[p])
                    va = pv.tile([128, NT, 65], bf16, tag="va", name="va")
                    nc.sync.dma_start(va[:],
                                      VA[p].rearrange("pp (t d) -> pp t d", d=65))
                    ps_sa = pss.tile([128, 65], f32, tag="sacc", name="ps_sa")
                    st[p] = dict(ktt=ktt, va=va, nr2=nr2, k2sb=k2sb,
                                 ps_sa=ps_sa, ers=[None] * 8)

                def defer_prefix(p):
                    g, jg = p // GP, p % GP
                    k2sb = st[p]["k2sb"]
                    ps_y0 = psn.tile([128, M], f32, tag="ns", name="ps_y0")
                    nc.tensor.matmul(ps_y0[:], k2sb[:], k2sb[:],
                                     start=True, stop=True)
                    nc.vector.tensor_scalar_mul(Ypk[g][:, jg, :], ps_y0[:],
                                                gsb[:])

                def er_block(p, blk):
                    ktt, nr2 = st[p]["ktt"], st[p]["nr2"]
                    ps_e = pse.tile([128, 8, 128], f32, tag="er", name="ps_e")
                    for c8 in range(8):
                        c = blk * 8 + c8
                        half = (c & 1) * 64
                        col = (c >> 1) * 128
                        slot = (c8 & 1) * 4 + (c8 >> 1)
                        nc.tensor.matmul(ps_e[:, slot, :],
                            ktt[half:half + 64, col:col + 128],
                            nr2[half:half + 64, :], start=True, stop=True)
                    ert = erb.tile([128, 8, 128], bf16, tag="erb", name="ert")
                    nc.scalar.activation(ert[:], ps_e[:], AF.Exp)
                    st[p]["ers"][blk] = ert

                def s_block(p, blk):
                    for c8 in range(8):
                        c = blk * 8 + c8
                        t_idx = (c & 1) * 32 + (c >> 1)
                        slot = (c8 & 1) * 4 + (c8 >> 1)
                        nc.tensor.matmul(st[p]["ps_sa"][:],
                            st[p]["ers"][blk][:, slot, :], st[p]["va"][:, t_idx, :],
                            start=(c == 0), stop=(c == 63),
                            skip_group_check=True)
                    st[p]["ers"][blk] = None

                def suffix(p):
                    g, jg = p // GP, p % GP
                    ns_state[f"adone{p}"] = True
                    ps_sa, k2sb = st[p]["ps_sa"], st[p]["k2sb"]
                    rr = sm2.tile([128, 1], f32, tag="rr", name="rr")
                    nc.vector.reciprocal(rr[:], ps_sa[:, 64:65])
                    rr2 = sm2.tile([128, 1], f32, tag="rr2", name="rr2")
                    nc.vector.tensor_tensor(out=rr2[:], in0=rr[:], in1=gsb[:],
                                            op=MUL)
                    s_bf = sm2.tile([128, 64], f16, tag="sbf", name="sbf")
                    nc.vector.tensor_scalar_mul(s_bf[:], ps_sa[:, 0:64], rr2[:])
                    ps_p0 = psn.tile([128, 64], f32, tag="ns", name="ps_p0")
                    nc.tensor.matmul(ps_p0[:], k2sb[:], s_bf[:],
                                     start=True, stop=True)
                    nc.vector.tensor_copy(Ppk[g][:, jg, :], ps_p0[:])

                LAG = 3
                head(0)
                blocks = [(p, b) for p in range(PAIRS) for b in range(8)]
                done_s = 0
                for idx, (p, b) in enumerate(blocks):
                    er_block(p, b)
                    if b == 1:
                        defer_prefix(p)
                    if b == 4 and p + 1 < PAIRS:
                        head(p + 1)
                    if p == 5 and b == 2:
                        c_prefetch(0)
                    if idx >= LAG:
                        sp, sb = blocks[done_s]
                        s_block(sp, sb)
                        done_s += 1
                        if sb == 7:
                            suffix(sp)
                    yield
                while done_s < len(blocks):
                    sp, sb = blocks[done_s]
                    s_block(sp, sb)
                    done_s += 1
                    if sb == 7:
                        suffix(sp)
                    yield

            def stage_b(g, psB):
                Ycur = ns_state["Y"][g]
                Pcur = ns_state["P"][g]
                for it in range(6):
                    ps_a = psB.tile([128, GP, 128], f32, tag="ns", name=f"psa{g}")
                    for j in range(GP):
                        nc.tensor.matmul(ps_a[:, j, :], Ycur[:, j, :],
                                         Ycur[:, j, :], start=True, stop=True)
                    yield
                    Rt = nsb.tile([128, GP, 128], f16, tag=f"R{g}", name=f"R{g}")
                    nc.vector.scalar_tensor_tensor(out=Rt[:], in0=ps_a[:],
                        scalar=-1.0 / 7.0, in1=Ycur[:], op0=MUL, op1=ADD)
                    yield
                    ps_q = psB.tile([128, GP, 128], f32, tag="ns", name=f"psq{g}")
                    for j in range(GP):
                        nc.tensor.matmul(ps_q[:, j, :], Ycur[:, j, :],
                                         Rt[:, j, :], start=True, stop=False)
                        nc.tensor.matmul(ps_q[:, j, :], dgt[:],
                                         Ycur[:, j, :], start=False, stop=True)
                    yield
                    qt_ = nsb.tile([128, GP, 128], f16, tag=f"q{g}", name=f"qm{g}")
                    nc.vector.scalar_tensor_tensor(out=qt_[:], in0=ps_q[:],
                        scalar=1.75, in1=i325[:], op0=MUL, op1=ADD)
                    yield
                    if it < 5:
                        ps_y = psB.tile([128, GP, 128], f32, tag="ns")
                        for j in range(GP):
                            nc.tensor.matmul(ps_y[:, j, :], qt_[:, j, :],
                                             Ycur[:, j, :], start=True, stop=True)
                        Ynew = nsb.tile([128, GP, 128], f16, tag=f"yn{g}",
                                        name=f"yn{g}")
                        nc.vector.tensor_copy(Ynew[:], ps_y[:])
                        Ycur = Ynew
                        yield
                    ps_p = psB.tile([128, GP, 64], f32, tag="ns")
                    for j in range(GP):
                        nc.tensor.matmul(ps_p[:, j, :], qt_[:, j, :],
                                         Pcur[:, j, :], start=True, stop=True)
                    Pnew = nsb.tile([128, GP, 64], f16, tag=f"pn{g}", name=f"pn{g}")
                    nc.vector.tensor_copy(Pnew[:], ps_p[:])
                    Pcur = Pnew
                    yield
                ns_state["P"][g] = Pcur
                for pp in range(GP):
                    p = g * GP + pp
                    bsb[p] = per.tile([128, 65], bf16, tag=f"b{p}", name=f"b{p}")
                    nc.vector.memset(bsb[p][:, 64:65], 1.0)
                    nc.vector.tensor_copy(bsb[p][:, 0:64], Pcur[:, pp, :])
                yield

            ecs_all = {}

            def c_prefetch(p):
                qtt[p] = pq.tile([128, NH], f16, tag=f"qt{p}", name=f"qt{p}")
                nc.sync.dma_start(qtt[p][:], QTT[p])
                nc2t[p] = pq.tile([128, M], f16, tag=f"nc2{p}", name=f"nc2{p}")
                nc.sync.dma_start(nc2t[p][:], NC2[p])

            def stage_c_pre(p, psE, delay=0):
                for _ in range(delay):
                    yield
                if p + 1 < PAIRS:
                    c_prefetch(p + 1)
                ecs = [None] * 8
                ecs_all[p] = ecs
                for sc in range(8):
                    ps_c = psE.tile([128, 2, 512], f32, tag="ec")
                    for t in range(2):
                        half = t * 64
                        nc.tensor.matmul(ps_c[:, t, :],
                            nc2t[p][half:half + 64, :],
                            qtt[p][half:half + 64, sc * 512:sc * 512 + 512],
                            start=True, stop=True)
                    ecs[sc] = ecb.tile([128, 2, 512], bf16, tag="ec", name="ect")
                    nc.scalar.activation(ecs[sc][:], ps_c[:], AF.Exp)
                    yield

            def stage_c_post(p, psx, act_share=0):
                ecs = ecs_all[p]
                xos = xob.tile([65, 2, NH], bf16, tag="xo")
                for sc in range(8):
                    ps_x = psx.tile([65, 2, 512], f32, tag="px", name="ps_x")
                    for t in range(2):
                        nc.tensor.matmul(ps_x[:, t, :], bsb[p][:],
                                         ecs[sc][:, t, :], start=True, stop=True)
                    dst = xos[:, :, sc * 512:sc * 512 + 512]
                    if sc >= 8 - act_share:
                        nc.scalar.copy(dst, ps_x[:])
                    else:
                        nc.vector.tensor_copy(dst, ps_x[:])
                    ecs[sc] = None
                    if sc == 3:
                        nc.sync.dma_start(XOT[p, :, :, 0:2048],
                                          xos[:, :, 0:2048])
                    yield
                nc.sync.dma_start(XOT[p, :, :, 2048:4096], xos[:, :, 2048:4096])
                yield

            def drain(*gens):
                live = list(gens)
                while live:
                    for gx in list(live):
                        try:
                            next(gx)
                        except StopIteration:
                            live.remove(gx)

            # schedule: A streams with B(0)/B(1) NS ladders zipped in as
            # their seed pairs complete; B(2) overlaps the first C-pre
            a = stage_a_all()
            live = [a]
            started = set()
            while live:
                for gx in list(live):
                    try:
                        next(gx)
                    except StopIteration:
                        live.remove(gx)
                for g in (0, 1):
                    if g not in started and ns_state.get(f"adone{g * GP + 1}"):
                        live.append(stage_b(g, psn))
                        started.add(g)
            assert started == {0, 1}
            psn.release()
            pss.release()
            pse.release()
            psE = tc.alloc_tile_pool(name="psE", bufs=2, space="PSUM")
            psB = tc.alloc_tile_pool(name="psB", bufs=2, space="PSUM")
            drain(stage_b(2, psB), stage_c_pre(0, psE, delay=4))
            psB.release()
            psx = tc.alloc_tile_pool(name="psx", bufs=2, space="PSUM")
            drain(stage_c_post(0, psx), stage_c_pre(1, psE))
            drain(stage_c_post(1, psx), stage_c_pre(2, psE))
            drain(stage_c_post(2, psx), stage_c_pre(3, psE))
            drain(stage_c_post(3, psx), stage_c_pre(4, psE))
            drain(stage_c_post(4, psx, act_share=2), stage_c_pre(5, psE))
            drain(stage_c_post(5, psx, act_share=6))
            psx.release()
            psE.release()
    nc.finalize()
    _cache["nc"] = nc
    return nc


def kernel(Q, K, V, mask):
    from concourse.bass_utils import run_bass_kernel_spmd

    Q = np.asarray(Q, dtype=np.float32)
    K = np.asarray(K, dtype=np.float32)
    V = np.asarray(V, dtype=np.float32)
    Qf = Q.reshape(B * H, N, D)
    Kf = K.reshape(B * H, N, D)
    Vf = V.reshape(B * H, N, D)

    nr = np.empty((B * H, M, D), np.float32)
    nc_ = np.empty((B * H, M, D), np.float32)
    K2h = np.empty((B * H, M, M), np.float16)
    gmax = 0.0
    for i in range(B * H):
        for (T, out) in ((Kf, nc_), (Qf, nr)):
            s = T[i, :, 0].copy()
            s[0] = np.inf
            idx = np.argpartition(-s, M)[:M]
            out[i] = T[i, np.sort(idx), :]
        m = nr[i].astype(np.float64) @ nc_[i].astype(np.float64).T
        e = np.exp(m - m.max(axis=1, keepdims=True))
        k2 = e / e.sum(axis=1, keepdims=True)
        K2h[i] = k2.astype(np.float16)
        gmax = max(gmax, float(k2.sum(axis=0).max()))

    def stack_halves(t16):  # [64, N] -> [128, N/2]
        return np.concatenate([t16[:, :NH], t16[:, NH:]], axis=0)

    QTTh = np.empty((B * H, 128, NH), np.float16)
    KTTh = np.empty((B * H, 128, NH), np.float16)
    for i in range(B * H):
        QTTh[i] = stack_halves(np.ascontiguousarray(Qf[i].T).astype(np.float16))
        KTTh[i] = stack_halves(np.ascontiguousarray(Kf[i].T).astype(np.float16))
    bf = ml_dtypes.bfloat16
    VAh = np.empty((B * H, 128, NT * 65), bf)
    ones = np.ones((N, 1), np.float32)
    for i in range(B * H):
        vaug = np.concatenate([Vf[i], ones], axis=1).astype(bf)  # [N, 65]
        VAh[i] = vaug.reshape(NT, 128, 65).transpose(1, 0, 2).reshape(128, NT * 65)
    nrt16 = np.ascontiguousarray(nr.transpose(0, 2, 1)).astype(np.float16)
    nct16 = np.ascontiguousarray(nc_.transpose(0, 2, 1)).astype(np.float16)
    NR2h = np.concatenate([nrt16, nrt16], axis=1)
    NC2h = np.concatenate([nct16, nct16], axis=1)
    gsv = np.array([[1.0 / gmax]], np.float32)

    ncb = _build()
    in_maps = []
    for c in range(NCORES):
        sl = slice(c * PAIRS, (c + 1) * PAIRS)
        in_maps.append({"QTT": QTTh[sl], "KTT": KTTh[sl], "VA": VAh[sl],
                        "NR2": NR2h[sl], "NC2": NC2h[sl],
                        "K2": K2h[sl], "GS": gsv})
    trace = os.environ.get("KERNEL_TRACE", "0") == "1"
    tmpdir = os.environ.get("KERNEL_TRACE_DIR") or None
    res = run_bass_kernel_spmd(ncb, in_maps, list(range(NCORES)),
                               trace=trace, tmpdir=tmpdir)
    global LAST_RESULTS
    LAST_RESULTS = res

    X = np.empty((B * H, N, D), np.float32)
    for c in range(NCORES):
        xot = np.asarray(res.results[c]["XOT"], dtype=np.float32)
        for pp in range(PAIRS):
            i = c * PAIRS + pp
            xf = xot[pp].reshape(65, N)
            X[i] = (xf[:64, :] / xf[64:65, :]).T
    return X.reshape(B, H, N, D)

